# revision 1
# baseline (speedup 1.0000x reference)
"""AUGRU (attention-modulated GRU) Trainium2 Bass kernel.

Problem: B=4096, T=200, D=H=128.  For each t:
  z = sigmoid([x,h] @ Wz + bz); r = sigmoid([x,h] @ Wr + br)
  h~ = tanh([x, r*h] @ Wh + bh); zp = a_t * z; h' = (1-zp)*h + zp*h~

Sharding: data-parallel over batch, B/8 = 512 rows per NeuronCore.

Host-side prep (inside kernel(), before dispatch): x is transposed to
[T, D, B_shard] and cast to bf16 (so the device streams it directly as
the matmul moving operand), attention scores to [T, B_shard] bf16, h0 to
[H, B_shard] bf16, weights split into x-part / h-part and cast bf16.

Per-core device layout: state hT [128(h), 512(b)] bf16 in SBUF.  The six
gate matmuls keep the 128x128 weights stationary and stream xT_t / hT /
(r*h)T with N=512 columns into PSUM (fp32 accumulate).  Biases enter as
K=1 matmuls (bias row stationary at partitions 0/32/64, a ones row
moving) opening the same PSUM accumulation groups.  sigmoid(z|r) is one
ACT instruction over a two-bank [128,1024] PSUM view; tanh a second.
The five per-step elementwise ops run on DVE in bf16; a_t is replicated
across partitions by GPSIMD partition_broadcast (idle engine).
"""

import numpy as np
import ml_dtypes

B, T, D, H = 4096, 200, 128, 128
NCORES = 8
BS = B // NCORES            # 512 batch rows per core
G = BS // 128               # 4 partition groups
C = 10                      # timestep chunk for x staging

BF = ml_dtypes.bfloat16

_compiled = None


def _build(t_steps=T, chunk=C, nchains=2):
    import concourse.bass as bass
    import concourse.bacc as bacc
    import concourse.mybir as mybir
    from concourse.tile import TileContext
    from concourse.bass_types import AP

    fp32 = mybir.dt.float32
    bf16 = mybir.dt.bfloat16
    Sigmoid = mybir.ActivationFunctionType.Sigmoid
    Tanh = mybir.ActivationFunctionType.Tanh

    nco = bacc.Bacc(
        "TRN2", target_bir_lowering=False, debug=False, num_devices=NCORES
    )
    xt_d = nco.dram_tensor("xt", [t_steps, D, BS], bf16, kind="ExternalInput")
    a_d = nco.dram_tensor("abf", [t_steps, BS], bf16, kind="ExternalInput")
    h0_d = nco.dram_tensor("h0t", [H, BS], bf16, kind="ExternalInput")
    wx_d = nco.dram_tensor("wx", [D, 3 * H], bf16, kind="ExternalInput")
    wh_d = nco.dram_tensor("wh", [H, 3 * H], bf16, kind="ExternalInput")
    # single partition row: per gate g, cols [g*(H+BS), g*(H+BS)+H) bias,
    # then BS ones (K=1 matmul stationary/moving operands)
    cb_d = nco.dram_tensor("cb", [1, 3 * (H + BS)], bf16, kind="ExternalInput")
    out_d = nco.dram_tensor("out", [H, BS], bf16, kind="ExternalOutput")

    with TileContext(nco) as tc:
        with (
            tc.tile_pool(name="const", bufs=1) as constp,
            tc.tile_pool(name="xT", bufs=6) as xTp,
            tc.tile_pool(name="ab", bufs=4) as abp,
            tc.tile_pool(name="state", bufs=2) as statep,
            tc.tile_pool(name="tmp", bufs=3) as tmpp,
            tc.tile_pool(name="zr", bufs=2) as zrp,
            tc.tile_pool(name="ps_zr", bufs=2, space="PSUM") as ps_zr,
            tc.tile_pool(name="ps_p", bufs=2, space="PSUM") as ps_p,
        ):
            mm = nco.tensor.matmul

            # ---- constants ----
            wx_sb = constp.tile([128, 3 * H], bf16, tag="wx")
            nco.sync.dma_start(out=wx_sb[:], in_=wx_d.ap())
            wh_sb = constp.tile([128, 3 * H], bf16, tag="wh")
            nco.sync.dma_start(out=wh_sb[:], in_=wh_d.ap())
            cb = constp.tile([1, 3 * (H + BS)], bf16, tag="cb")
            nco.gpsimd.dma_start(out=cb[:], in_=cb_d.ap())
            # prime the ACT engine's clock on cb so no activation ever
            # carries a second (const-DMA) wait
            actprime = constp.tile([1, 4], bf16, tag="actprime")
            nco.scalar.copy(actprime[:], cb[:, 0:4])

            CW = BS // nchains  # chain width (batch columns per chain)
            S = H + BS
            hTs = []
            for c in range(nchains):
                hT = statep.tile([128, CW], bf16, tag=f"h{c}")
                nco.sync.dma_start(
                    out=hT[:], in_=h0_d.ap()[:, c * CW : (c + 1) * CW]
                )
                hTs.append(hT[:])

            # Per-chain phase emitters.  P1: gate matmuls + sigmoid.
            # P2: r*h, h~ matmul, tanh, plus the off-path q = a*z.
            # P3: the short post-tanh tail d -> m -> h'.
            st = [dict(hT=hTs[c]) for c in range(nchains)]

            def p1(c, t, xT_t):
                s_ = st[c]
                cw = slice(c * CW, (c + 1) * CW)
                # a_t broadcast down the 128 partitions: DMA re-reads the
                # DRAM row with a zero-step outer dim
                ab = abp.tile([128, CW], bf16, tag=f"ab{c}")
                asrc = a_d.ap()[t : t + 1, cw]
                asrc = AP(asrc.tensor, asrc.offset, [[0, 128]] + list(asrc.ap[1:]))
                nco.gpsimd.dma_start(out=ab[:], in_=asrc)
                s_["ab"] = ab
                zr_ps = ps_zr.tile([128, 2 * CW], fp32, tag=f"zrps{c}")
                p_ps = ps_p.tile([128, CW], fp32, tag=f"pps{c}")
                # PE: bias rows open each PSUM group (const operands =>
                # only the slot-release wait), then x-parts, h-parts
                mm(zr_ps[:, 0:CW], cb[0:1, 0:H], cb[0:1, H : H + CW],
                   start=True, stop=False)
                mm(zr_ps[:, CW:], cb[0:1, S : S + H], cb[0:1, S + H : S + H + CW],
                   start=True, stop=False)
                mm(p_ps[:], cb[0:1, 2 * S : 2 * S + H], cb[0:1, 2 * S + H : 2 * S + H + CW],
                   start=True, stop=False)
                mm(zr_ps[:, 0:CW], wx_sb[:, 0:128], xT_t[:, cw], start=False, stop=False)
                mm(zr_ps[:, CW:], wx_sb[:, 128:256], xT_t[:, cw], start=False, stop=False)
                mm(p_ps[:], wx_sb[:, 256:384], xT_t[:, cw], start=False, stop=False)
                mm(zr_ps[:, 0:CW], wh_sb[:, 0:128], s_["hT"], start=False, stop=True)
                mm(zr_ps[:, CW:], wh_sb[:, 128:256], s_["hT"], start=False, stop=True)
                zr_bf = zrp.tile([128, 2 * CW], bf16, tag=f"zr{c}")
                nco.scalar.activation(zr_bf[:], zr_ps[:], Sigmoid)
                s_["zr_ps"], s_["p_ps"], s_["zr_bf"] = zr_ps, p_ps, zr_bf

            def p2(c):
                s_ = st[c]
                rh = tmpp.tile([128, CW], bf16, tag=f"rh{c}")
                nco.vector.tensor_mul(rh[:], s_["zr_bf"][:, CW:], s_["hT"])
                # off the critical path: q = a * z
                qq = tmpp.tile([128, CW], bf16, tag=f"q{c}")
                nco.vector.tensor_mul(qq[:], s_["ab"][:], s_["zr_bf"][:, 0:CW])
                mm(s_["p_ps"][:], wh_sb[:, 256:384], rh[:], start=False, stop=True)
                ht_ = tmpp.tile([128, CW], bf16, tag=f"ht{c}")
                nco.scalar.activation(ht_[:], s_["p_ps"][:], Tanh)
                s_["q"], s_["ht"] = qq, ht_

            def p3(c):
                s_ = st[c]
                dd = tmpp.tile([128, CW], bf16, tag=f"d{c}")
                nco.vector.tensor_sub(dd[:], s_["ht"][:], s_["hT"])
                mt = tmpp.tile([128, CW], bf16, tag=f"m{c}")
                nco.vector.tensor_mul(mt[:], s_["q"][:], dd[:])
                hT_new = statep.tile([128, CW], bf16, tag=f"h{c}")
                nco.vector.tensor_add(hT_new[:], s_["hT"], mt[:])
                hTs[c] = hT_new[:]
                s_["hT"] = hTs[c]

            def xload(t):
                # x_t [128(d), BS] — one DMA per step so consumers wait on
                # a single semaphore (large sprayed DMAs overflow the ISA
                # wait-slot budget of the consuming matmul)
                xT_t = xTp.tile([128, BS], bf16, tag="xT")
                nco.sync.dma_start(out=xT_t[:], in_=xt_d.ap()[t])
                return xT_t[:]

            if nchains == 1:
                for t in range(t_steps):
                    xT_t = xload(t)
                    p1(0, t, xT_t)
                    p2(0)
                    p3(0)
            else:
                # Software-pipelined half-step stagger: chain B runs half a
                # step behind A so each engine's in-order stream alternates
                # between a stalled chain and a ready one.
                for t in range(t_steps):
                    xT_t = xload(t)
                    p1(0, t, xT_t)
                    if t > 0:
                        p2(1)
                    p2(0)
                    if t > 0:
                        p3(1)
                    p1(1, t, xT_t)
                    p3(0)
                p2(1)
                p3(1)

            # ---- store final state transposed [H, BS] bf16; host flips ----
            for c in range(nchains):
                nco.gpsimd.dma_start(
                    out=out_d.ap()[:, c * CW : (c + 1) * CW], in_=hTs[c]
                )

    nco.compile()
    return nco


def _in_maps(inputs, t_steps=T):
    x = np.asarray(inputs["inputs"], np.float32)
    a = np.asarray(inputs["attention_scores"], np.float32)
    h0 = np.asarray(inputs["h0"], np.float32)
    Wz = np.asarray(inputs["Wz"], np.float32)
    Wr = np.asarray(inputs["Wr"], np.float32)
    Wh = np.asarray(inputs["Wh"], np.float32)
    wx = np.concatenate([Wz[:D], Wr[:D], Wh[:D]], axis=1).astype(BF)
    wh = np.concatenate([Wz[D:], Wr[D:], Wh[D:]], axis=1).astype(BF)
    cb = np.ones((1, 3 * (H + BS)), np.float32)  # cast to bf16 below
    for i, k in enumerate(("bz", "br", "bh")):
        cb[0, i * (H + BS) : i * (H + BS) + H] = np.asarray(inputs[k], np.float32)
    cb = cb.astype(BF)
    maps = []
    for c in range(NCORES):
        sl = slice(c * BS, (c + 1) * BS)
        maps.append(
            {
                # [T, D, BS] bf16: host transpose + cast
                "xt": np.ascontiguousarray(
                    x[sl, :t_steps].transpose(1, 2, 0)
                ).astype(BF),
                "abf": np.ascontiguousarray(a[sl, :t_steps].T).astype(BF),
                "h0t": np.ascontiguousarray(h0[sl].T).astype(BF),
                "wx": wx,
                "wh": wh,
                "cb": cb,
            }
        )
    return maps


def kernel(**inputs):
    global _compiled
    from concourse.bass_utils import run_bass_kernel_spmd

    if _compiled is None:
        _compiled = _build()
    res = run_bass_kernel_spmd(_compiled, _in_maps(inputs), core_ids=list(range(NCORES)))
    return np.ascontiguousarray(
        np.concatenate(
            [np.asarray(r["out"]).astype(np.float32).T for r in res.results], axis=0
        )
    )



# revision 4
# speedup vs baseline: 1.2640x; 1.2640x over previous
"""AUGRU (attention-modulated GRU) Trainium2 Bass kernel.

Problem: B=4096, T=200, D=H=128.  For each t:
  z = sigmoid([x,h] @ Wz + bz); r = sigmoid([x,h] @ Wr + br)
  h~ = tanh([x, r*h] @ Wh + bh); zp = a_t * z; h' = (1-zp)*h + zp*h~

Sharding: data-parallel over batch, B/8 = 512 rows per NeuronCore.

Host-side prep (inside kernel(), before dispatch): x is transposed to
[T, D, B_shard] and cast to bf16 (so the device streams it directly as
the matmul moving operand), attention scores to [T, B_shard] bf16, h0 to
[H, B_shard] bf16, weights split into x-part / h-part and cast bf16.

Per-core device layout: state hT [128(h), 512(b)] bf16 in SBUF.  The six
gate matmuls keep the 128x128 weights stationary and stream xT_t / hT /
(r*h)T with N=512 columns into PSUM (fp32 accumulate).  Biases are folded
into the activations via the ACT engine's per-partition bias operand
(bias indexes h = the partition dim in this layout), so no bias matmuls.
sigmoid r / sigmoid z are separate ACTs (r first: the candidate path
r -> r*h -> matmul -> tanh is the long pole).  a_t is replicated across
partitions by a chunked zero-stride DMA; q = a*z runs on GPSIMD (idle),
the remaining elementwise ops on DVE in bf16.
"""

import numpy as np
import ml_dtypes

B, T, D, H = 4096, 200, 128, 128
NCORES = 8
BS = B // NCORES            # 512 batch rows per core
G = BS // 128               # 4 partition groups
C = 10                      # timestep chunk for attention staging

BF = ml_dtypes.bfloat16

_compiled = None


def _build(t_steps=T, chunk=C, nchains=2):
    import concourse.bass as bass
    import concourse.bacc as bacc
    import concourse.mybir as mybir
    from concourse.tile import TileContext
    from concourse.bass_types import AP

    fp32 = mybir.dt.float32
    bf16 = mybir.dt.bfloat16
    Sigmoid = mybir.ActivationFunctionType.Sigmoid
    Tanh = mybir.ActivationFunctionType.Tanh

    assert t_steps % chunk == 0
    nchunks = t_steps // chunk

    nco = bacc.Bacc(
        "TRN2", target_bir_lowering=False, debug=False, num_devices=NCORES
    )
    xt_d = nco.dram_tensor("xt", [t_steps, D, BS], bf16, kind="ExternalInput")
    a_d = nco.dram_tensor("abf", [t_steps, BS], bf16, kind="ExternalInput")
    h0_d = nco.dram_tensor("h0t", [H, BS], bf16, kind="ExternalInput")
    wx_d = nco.dram_tensor("wx", [D, 3 * H], bf16, kind="ExternalInput")
    wh_d = nco.dram_tensor("wh", [H, 3 * H], bf16, kind="ExternalInput")
    b_d = nco.dram_tensor("bcol", [H, 4], fp32, kind="ExternalInput")
    out_d = nco.dram_tensor("out", [H, BS], bf16, kind="ExternalOutput")

    with TileContext(nco) as tc:
        with (
            tc.tile_pool(name="const", bufs=1) as constp,
            tc.tile_pool(name="xT", bufs=6) as xTp,
            tc.tile_pool(name="ab", bufs=2) as abp,
            tc.tile_pool(name="state", bufs=2) as statep,
            tc.tile_pool(name="tmp", bufs=3) as tmpp,
            tc.tile_pool(name="zr", bufs=2) as zrp,
            tc.tile_pool(name="ps_zr", bufs=2, space="PSUM") as ps_zr,
            tc.tile_pool(name="ps_p", bufs=2, space="PSUM") as ps_p,
        ):
            mm = nco.tensor.matmul

            # ---- constants ----
            wx_sb = constp.tile([128, 3 * H], bf16, tag="wx")
            nco.sync.dma_start(out=wx_sb[:], in_=wx_d.ap())
            wh_sb = constp.tile([128, 3 * H], bf16, tag="wh")
            nco.sync.dma_start(out=wh_sb[:], in_=wh_d.ap())
            b_sb = constp.tile([128, 4], fp32, tag="bcol")
            nco.sync.dma_start(out=b_sb[:], in_=b_d.ap())

            CW = BS // nchains  # chain width (batch columns per chain)
            hTs = []
            for c in range(nchains):
                hT = statep.tile([128, CW], bf16, tag=f"h{c}")
                nco.sync.dma_start(
                    out=hT[:], in_=h0_d.ap()[:, c * CW : (c + 1) * CW]
                )
                hTs.append(hT[:])

            # Chunked attention broadcast: one zero-stride DMA replicates
            # a[t0:t0+chunk, :] across all 128 partitions.
            def abload(t0):
                ab_ch = abp.tile([128, chunk, BS], bf16, tag="ab")
                asrc = a_d.ap()[t0 : t0 + chunk, :]
                asrc = AP(asrc.tensor, asrc.offset, [[0, 128]] + list(asrc.ap))
                nco.sync.dma_start(out=ab_ch[:], in_=asrc)
                return ab_ch

            ab_chunks = [None] * nchunks
            ab_chunks[0] = abload(0)
            if nchunks > 1:
                ab_chunks[1] = abload(chunk)

            # Per-chain phase emitters.  P1: gate matmuls + sigmoids.
            # P2: r*h, h~ matmul, tanh, plus the off-path q = a*z on GPSIMD.
            # P3: the short post-tanh tail d -> m -> h'.
            st = [dict(hT=hTs[c]) for c in range(nchains)]

            def p1(c, t, xT_t):
                s_ = st[c]
                cw = slice(c * CW, (c + 1) * CW)
                s_["ab"] = ab_chunks[t // chunk][:, t % chunk, cw]
                zr_ps = ps_zr.tile([128, 2 * CW], fp32, tag=f"zrps{c}")
                p_ps = ps_p.tile([128, CW], fp32, tag=f"pps{c}")
                # x-parts open the PSUM groups (no bias matmuls: biases are
                # folded into the ACT bias operand below)
                mm(zr_ps[:, CW:], wx_sb[:, 128:256], xT_t[:, cw], start=True, stop=False)
                mm(zr_ps[:, 0:CW], wx_sb[:, 0:128], xT_t[:, cw], start=True, stop=False)
                mm(p_ps[:], wx_sb[:, 256:384], xT_t[:, cw], start=True, stop=False)
                # h-parts close z|r; r first (the candidate path needs it)
                mm(zr_ps[:, CW:], wh_sb[:, 128:256], s_["hT"], start=False, stop=True)
                mm(zr_ps[:, 0:CW], wh_sb[:, 0:128], s_["hT"], start=False, stop=True)
                zr_bf = zrp.tile([128, 2 * CW], bf16, tag=f"zr{c}")
                nco.scalar.activation(
                    zr_bf[:, CW:], zr_ps[:, CW:], Sigmoid, bias=b_sb[:, 1:2]
                )
                nco.scalar.activation(
                    zr_bf[:, 0:CW], zr_ps[:, 0:CW], Sigmoid, bias=b_sb[:, 0:1]
                )
                s_["zr_ps"], s_["p_ps"], s_["zr_bf"] = zr_ps, p_ps, zr_bf

            def p2(c):
                s_ = st[c]
                rh = tmpp.tile([128, CW], bf16, tag=f"rh{c}")
                nco.vector.tensor_mul(rh[:], s_["zr_bf"][:, CW:], s_["hT"])
                # off the critical path: q = a * z on GPSIMD
                qq = tmpp.tile([128, CW], bf16, tag=f"q{c}")
                nco.gpsimd.tensor_mul(qq[:], s_["ab"], s_["zr_bf"][:, 0:CW])
                mm(s_["p_ps"][:], wh_sb[:, 256:384], rh[:], start=False, stop=True)
                ht_ = tmpp.tile([128, CW], bf16, tag=f"ht{c}")
                nco.scalar.activation(
                    ht_[:], s_["p_ps"][:], Tanh, bias=b_sb[:, 2:3]
                )
                s_["q"], s_["ht"] = qq, ht_

            def p3(c):
                s_ = st[c]
                dd = tmpp.tile([128, CW], bf16, tag=f"d{c}")
                nco.vector.tensor_sub(dd[:], s_["ht"][:], s_["hT"])
                mt = tmpp.tile([128, CW], bf16, tag=f"m{c}")
                nco.vector.tensor_mul(mt[:], s_["q"][:], dd[:])
                hT_new = statep.tile([128, CW], bf16, tag=f"h{c}")
                nco.vector.tensor_add(hT_new[:], s_["hT"], mt[:])
                hTs[c] = hT_new[:]
                s_["hT"] = hTs[c]

            def xload(t):
                # x_t [128(d), BS] — one DMA per step so consumers wait on
                # a single semaphore (large sprayed DMAs overflow the ISA
                # wait-slot budget of the consuming matmul)
                xT_t = xTp.tile([128, BS], bf16, tag="xT")
                nco.sync.dma_start(out=xT_t[:], in_=xt_d.ap()[t])
                return xT_t[:]

            if nchains == 1:
                for t in range(t_steps):
                    nxt = t // chunk + 1
                    if t % chunk == 0 and nxt < nchunks and ab_chunks[nxt] is None:
                        ab_chunks[nxt] = abload(t + chunk)
                    xT_t = xload(t)
                    p1(0, t, xT_t)
                    p2(0)
                    p3(0)
            else:
                # Software-pipelined half-step stagger: chain B runs half a
                # step behind A so each engine's in-order stream alternates
                # between a stalled chain and a ready one.
                for t in range(t_steps):
                    nxt = t // chunk + 1
                    if t % chunk == 0 and nxt < nchunks and ab_chunks[nxt] is None:
                        ab_chunks[nxt] = abload(t + chunk)
                    xT_t = xload(t)
                    p1(0, t, xT_t)
                    if t > 0:
                        p2(1)
                    p2(0)
                    if t > 0:
                        p3(1)
                    p1(1, t, xT_t)
                    p3(0)
                p2(1)
                p3(1)

            # ---- store final state transposed [H, BS] bf16; host flips ----
            for c in range(nchains):
                nco.gpsimd.dma_start(
                    out=out_d.ap()[:, c * CW : (c + 1) * CW], in_=hTs[c]
                )

    nco.compile()
    return nco


def _in_maps(inputs, t_steps=T):
    x = np.asarray(inputs["inputs"], np.float32)
    a = np.asarray(inputs["attention_scores"], np.float32)
    h0 = np.asarray(inputs["h0"], np.float32)
    Wz = np.asarray(inputs["Wz"], np.float32)
    Wr = np.asarray(inputs["Wr"], np.float32)
    Wh = np.asarray(inputs["Wh"], np.float32)
    wx = np.concatenate([Wz[:D], Wr[:D], Wh[:D]], axis=1).astype(BF)
    wh = np.concatenate([Wz[D:], Wr[D:], Wh[D:]], axis=1).astype(BF)
    bcol = np.zeros((H, 4), np.float32)
    for i, k in enumerate(("bz", "br", "bh")):
        bcol[:, i] = np.asarray(inputs[k], np.float32)
    maps = []
    for c in range(NCORES):
        sl = slice(c * BS, (c + 1) * BS)
        maps.append(
            {
                # [T, D, BS] bf16: host transpose + cast
                "xt": np.ascontiguousarray(
                    x[sl, :t_steps].transpose(1, 2, 0)
                ).astype(BF),
                "abf": np.ascontiguousarray(a[sl, :t_steps].T).astype(BF),
                "h0t": np.ascontiguousarray(h0[sl].T).astype(BF),
                "wx": wx,
                "wh": wh,
                "bcol": bcol,
            }
        )
    return maps


def kernel(**inputs):
    global _compiled
    from concourse.bass_utils import run_bass_kernel_spmd

    if _compiled is None:
        _compiled = _build()
    res = run_bass_kernel_spmd(_compiled, _in_maps(inputs), core_ids=list(range(NCORES)))
    return np.ascontiguousarray(
        np.concatenate(
            [np.asarray(r["out"]).astype(np.float32).T for r in res.results], axis=0
        )
    )
